# revision 7
# baseline (speedup 1.0000x reference)
"""Trainium2 Bass kernel for nn_CLLayer_47064251630125 (contrastive loss).

Reference computation (per row i of N=8192):
    h1 = ELU(z1 @ W1.T + b1) @ W2.T + b2 ; h2 likewise
    na = normalize(h1), nb = normalize(h2)   (L2 row norm)
    l1 = -log( exp(2 na_i.nb_i) / (sum_j exp(2 na_i.na_j) + sum_j exp(2 na_i.nb_j) - e^2) )
    l2 = same with roles swapped (uses column sums of the cross matrix)
    out = (l1 + l2)/2

Sharding: each core owns a 1024-row block.  The host passes, per core, the
full transposed projections' inputs [256, 8192] with that core's 1024-col
block APPENDED (cols 8192..9215) -> one SPMD NEFF, no collectives, no
per-core code.  Each core computes row sums of exp(similarity) for 4
streams (R1, B, BT, R2) over [block x full] via PE matmuls + fused
exp/row-sum on the scalar engine (activation accum_out).
"""

import sys

sys.path.insert(0, "/opt/trn_rl_repo")

import numpy as np
import ml_dtypes

import concourse.bass as bass
import concourse.mybir as mybir
import concourse.tile as tile
from concourse import bacc

BF16 = mybir.dt.bfloat16
F32 = mybir.dt.float32
AF = mybir.ActivationFunctionType
ALU = mybir.AluOpType

P = 128
D = 256
KT = D // P          # 2 k-tiles
N_FULL = 8192
N_CORES = 8
TAU = 0.5
SIM_SCALE = 1.0 / TAU      # 2.0
E2 = float(np.exp(SIM_SCALE))  # exp(2 * ||na||^2) ~ e^2, diag of refl


def build_bass(n_full=N_FULL, blk=None, n_cores=N_CORES):
    """Trace the Tile kernel.  Returns the Bacc object (one NEFF, SPMD)."""
    if blk is None:
        blk = n_full // n_cores
    W = n_full + blk              # projected columns per tensor
    CH = 512                      # projection chunk (free dim per matmul)
    NCH = W // CH
    JG = 2048                     # similarity exp-group width (4 psum banks)
    NJG = n_full // JG
    ISUB = blk // P               # i-subtiles per core block

    nc = bacc.Bacc("TRN2", target_bir_lowering=False, debug=False,
                   num_devices=n_cores)

    z1t = nc.dram_tensor("z1t", [D, W], BF16, kind="ExternalInput")
    z2t = nc.dram_tensor("z2t", [D, W], BF16, kind="ExternalInput")
    w1t = nc.dram_tensor("w1t", [D, D], BF16, kind="ExternalInput")
    w2t = nc.dram_tensor("w2t", [D, D], BF16, kind="ExternalInput")
    b1d = nc.dram_tensor("b1", [D], BF16, kind="ExternalInput")
    b2d = nc.dram_tensor("b2", [D], BF16, kind="ExternalInput")
    out = nc.dram_tensor("out", [P, ISUB], F32, kind="ExternalOutput")

    with tile.TileContext(nc) as tc:
        with (
            tc.tile_pool(name="const", bufs=1) as cpool,
            tc.tile_pool(name="persist", bufs=1) as ppool,
            tc.tile_pool(name="io", bufs=3) as iopool,
            tc.tile_pool(name="scratch", bufs=3) as spool,
            tc.tile_pool(name="dram", bufs=2, space="DRAM") as dpool,
        ):
            # ---- constants ----
            w1_sb = cpool.tile([P, KT, D], BF16)
            nc.sync.dma_start(w1_sb, w1t.rearrange("(k p) c -> p k c", p=P))
            w2_sb = cpool.tile([P, KT, D], BF16)
            nc.sync.dma_start(w2_sb, w2t.rearrange("(k p) c -> p k c", p=P))
            b1_sb = cpool.tile([1, D], BF16)
            nc.sync.dma_start(b1_sb, b1d[None, :])
            b2_sb = cpool.tile([1, D], BF16)
            nc.sync.dma_start(b2_sb, b2d[None, :])
            ones_col = cpool.tile([P, 1], BF16)
            nc.vector.memset(ones_col, 1.0)
            ones_row = cpool.tile([1, CH], BF16)
            nc.vector.memset(ones_row, 1.0)

            na_bufs = []
            for idx, zt in enumerate((z1t, z2t)):
                zt_ap = zt.rearrange("(k p) w -> p k w", p=P)
                naT = ppool.tile([P, KT, W], BF16, name=f"naT{idx}",
                                 tag=f"naT{idx}")
                hT = ppool.tile([P, KT, W], BF16, name=f"hT{idx}", tag="hT")
                nsq = ppool.tile([NCH, CH], F32, name=f"nsq{idx}", tag="nsq")
                rn_dram = dpool.tile([NCH, CH], BF16, name=f"rn{idx}")

                # ---- phase A: project, ELU, square, col norms ----
                with tc.tile_pool(name="psA", bufs=2, space="PSUM") as psA, \
                     tc.tile_pool(name="psN", bufs=2, space="PSUM") as psN:
                    for c in range(NCH):
                        cs = slice(c * CH, (c + 1) * CH)
                        zch = iopool.tile([P, KT, CH], BF16, tag="zch")
                        nc.sync.dma_start(zch, zt_ap[:, :, cs])
                        # L1: pa[m] = W1 @ z.T + b1  (b1 as K=1 row)
                        pa = psA.tile([P, KT, CH], F32, tag="pa")
                        for m in range(KT):
                            ms = slice(m * P, (m + 1) * P)
                            for k in range(KT):
                                nc.tensor.matmul(pa[:, m], w1_sb[:, k, ms],
                                                 zch[:, k], start=(k == 0),
                                                 stop=False)
                            nc.tensor.matmul(pa[:, m], b1_sb[:, ms],
                                             ones_row, start=False, stop=True)
                        # ELU' = elu+1 = min(exp(x), relu(x)+1)
                        # (the -1 is folded into b2 on the host)
                        e_t = spool.tile([P, KT, CH], BF16, tag="e")
                        r_t = spool.tile([P, KT, CH], BF16, tag="r")
                        aT = spool.tile([P, KT, CH], BF16, tag="aT")
                        for m in range(KT):
                            nc.scalar.activation(e_t[:, m], pa[:, m], AF.Exp)
                            nc.vector.tensor_scalar(r_t[:, m], pa[:, m],
                                                    0.0, 1.0, ALU.max, ALU.add)
                            nc.vector.tensor_tensor(aT[:, m], e_t[:, m],
                                                    r_t[:, m], ALU.min)
                        # L2: ph[m2] = W2 @ a + b2_eff
                        ph = psA.tile([P, KT, CH], F32, tag="ph", bufs=1)
                        for m2 in range(KT):
                            ms = slice(m2 * P, (m2 + 1) * P)
                            for m in range(KT):
                                nc.tensor.matmul(ph[:, m2], w2_sb[:, m, ms],
                                                 aT[:, m], start=(m == 0),
                                                 stop=False)
                            nc.tensor.matmul(ph[:, m2], b2_sb[:, ms],
                                             ones_row, start=False, stop=True)
                        # copy h to sbuf (bf16), square on pool, col sums on PE
                        sq = spool.tile([P, KT, CH], BF16, tag="sq")
                        for m2 in range(KT):
                            nc.vector.tensor_copy(hT[:, m2, cs], ph[:, m2])
                            nc.vector.tensor_tensor(sq[:, m2], hT[:, m2, cs],
                                                    hT[:, m2, cs], ALU.mult)
                        ns = psN.tile([1, CH], F32, tag="ns")
                        for m2 in range(KT):
                            nc.tensor.matmul(ns, ones_col, sq[:, m2],
                                             start=(m2 == 0),
                                             stop=(m2 == KT - 1))
                        ns_sb = spool.tile([1, CH], F32, tag="ns_sb")
                        nc.vector.tensor_copy(ns_sb, ns)
                        nc.sync.dma_start(nsq[c:c + 1, :], ns_sb)

                # ---- phase B: rn = 1/sqrt(nsq) ----
                rcp = ppool.tile([NCH, CH], F32, name=f"rcp{idx}", tag="rcp")
                nc.vector.reciprocal(rcp, nsq)
                rn_sb = ppool.tile([NCH, CH], BF16, name=f"rnsb{idx}",
                                   tag="rnsb")
                nc.scalar.activation(rn_sb, rcp, AF.Sqrt)
                nc.sync.dma_start(rn_dram, rn_sb)

                # ---- phase C: naT = hT * rn (broadcast over partitions) ----
                for c in range(NCH):
                    cs = slice(c * CH, (c + 1) * CH)
                    rnB = spool.tile([P, CH], BF16, tag="rnB")
                    nc.sync.dma_start(
                        rnB, rn_dram[c:c + 1, :].to_broadcast([P, CH]))
                    for k in range(KT):
                        nc.vector.tensor_tensor(naT[:, k, cs], hT[:, k, cs],
                                                rnB, ALU.mult)
                na_bufs.append(naT)

            naT, nbT = na_bufs

            # ---- positive-pair dots: pos[i] = na_i . nb_i  (block cols) ----
            fin = ppool.tile([P, 10, ISUB], F32)  # rows of final scratch
            with tc.tile_pool(name="psP", bufs=1, space="PSUM") as psP:
                pd = spool.tile([P, KT, blk], BF16, tag="pd")
                for k in range(KT):
                    nc.vector.tensor_tensor(pd[:, k], naT[:, k, n_full:],
                                            nbT[:, k, n_full:], ALU.mult)
                pos_ps = psP.tile([P, ISUB], F32)
                for s in range(ISUB):
                    ss = slice(s * P, (s + 1) * P)
                    for k in range(KT):
                        nc.tensor.matmul(pos_ps[:, s:s + 1], pd[:, k, ss],
                                         ones_col, start=(k == 0),
                                         stop=(k == KT - 1))
                nc.vector.tensor_copy(fin[:, 9], pos_ps)

            # ---- similarity streams: row sums of exp(2 * sim) ----
            streams = [(naT, naT), (naT, nbT), (nbT, naT), (nbT, nbT)]
            rs = ppool.tile([P, 4, ISUB * NJG], F32)
            with tc.tile_pool(name="psS", bufs=2, space="PSUM") as psS:
                for st, (lhs_b, rhs_b) in enumerate(streams):
                    for isub in range(ISUB):
                        lslice = slice(n_full + isub * P,
                                       n_full + (isub + 1) * P)
                        for jg in range(NJG):
                            pg = psS.tile([P, JG // CH, CH], F32, tag="sgrp")
                            for k in range(KT):
                                for js in range(JG // CH):
                                    jss = slice(jg * JG + js * CH,
                                                jg * JG + (js + 1) * CH)
                                    nc.tensor.matmul(
                                        pg[:, js], lhs_b[:, k, lslice],
                                        rhs_b[:, k, jss], start=(k == 0),
                                        stop=(k == KT - 1))
                            eg = spool.tile([P, JG // CH, CH], BF16, tag="eg")
                            nc.scalar.activation(
                                eg, pg, AF.Exp, scale=SIM_SCALE,
                                accum_out=rs[:, st, isub * NJG + jg:
                                             isub * NJG + jg + 1])

            # ---- final: l = 0.5*(ln d1 + ln d2) - 2*pos ----
            rs4 = rs.rearrange("p s (i j) -> p s i j", j=NJG)
            for st in range(4):
                nc.vector.tensor_reduce(out=fin[:, st, :, None],
                                        in_=rs4[:, st], op=ALU.add,
                                        axis=mybir.AxisListType.X)
            # d1 = rsR1 + rsB - e2 ; d2 = rsR2 + rsBT - e2
            nc.vector.scalar_tensor_tensor(fin[:, 4], fin[:, 0], -E2,
                                           fin[:, 1], ALU.add, ALU.add)
            nc.vector.scalar_tensor_tensor(fin[:, 5], fin[:, 3], -E2,
                                           fin[:, 2], ALU.add, ALU.add)
            nc.scalar.activation(fin[:, 6], fin[:, 4], AF.Ln)
            nc.scalar.activation(fin[:, 7], fin[:, 5], AF.Ln)
            nc.vector.tensor_tensor(fin[:, 8], fin[:, 6], fin[:, 7], ALU.add)
            # l = 0.5 * (lnd1 + lnd2 - 2*SIM_SCALE*pos)
            lres = ppool.tile([P, ISUB], F32)
            nc.vector.scalar_tensor_tensor(lres, fin[:, 9],
                                           -2.0 * SIM_SCALE, fin[:, 8],
                                           ALU.mult, ALU.add)
            nc.vector.tensor_scalar_mul(lres, lres, 0.5)
            nc.sync.dma_start(out[:, :], lres)

    nc.compile()
    return nc


def prep_inputs(z1, z2, W1, b1, W2, b2, n_full=N_FULL, n_cores=N_CORES):
    """Host-side prep -> list of per-core input maps (numpy)."""
    blk = n_full // n_cores
    bf = ml_dtypes.bfloat16
    z1t = np.ascontiguousarray(z1.T)
    z2t = np.ascontiguousarray(z2.T)
    w1t = np.ascontiguousarray(W1.T).astype(bf)
    w2t = np.ascontiguousarray(W2.T).astype(bf)
    # ELU' = elu + 1 is used as the L1 activation; fold the "-1" into b2:
    # h = W2 @ (elu'(x) - 1) + b2 = W2 @ elu'(x) + (b2 - W2.sum(1))
    b2_eff = (b2 - W2.sum(axis=1)).astype(bf)
    b1c = b1.astype(bf)
    in_maps = []
    for c in range(n_cores):
        bs = slice(c * blk, (c + 1) * blk)
        in_maps.append({
            "z1t": np.concatenate([z1t, z1t[:, bs]], axis=1).astype(bf),
            "z2t": np.concatenate([z2t, z2t[:, bs]], axis=1).astype(bf),
            "w1t": w1t, "w2t": w2t, "b1": b1c, "b2": b2_eff,
        })
    return in_maps


_NC_CACHE = {}


def _get_nc(n_full=N_FULL, n_cores=N_CORES):
    key = (n_full, n_cores)
    if key not in _NC_CACHE:
        _NC_CACHE[key] = build_bass(n_full=n_full, n_cores=n_cores)
    return _NC_CACHE[key]


def kernel(z1, z2, W1, b1, W2, b2):
    from concourse.bass_utils import run_bass_kernel_spmd

    n_full = z1.shape[0]
    n_cores = N_CORES
    in_maps = prep_inputs(z1, z2, W1, b1, W2, b2, n_full, n_cores)
    nc = _get_nc(n_full, n_cores)
    res = run_bass_kernel_spmd(nc, in_maps, core_ids=list(range(n_cores)))
    parts = [np.asarray(res.results[c]["out"]).T.reshape(-1)
             for c in range(n_cores)]
    return np.concatenate(parts).astype(np.float32)


if __name__ == "__main__":
    nc = build_bass()
    print("traced ok:", len(nc.m.functions[0].blocks[0].instructions)
          if nc.m.functions[0].blocks else "n/a")


# revision 27
# speedup vs baseline: 1.5517x; 1.5517x over previous
"""Trainium2 Bass kernel for nn_CLLayer_47064251630125 (contrastive loss).

Reference computation (per row i of N=8192):
    h1 = ELU(z1 @ W1.T + b1) @ W2.T + b2 ; h2 likewise
    na = normalize(h1), nb = normalize(h2)   (L2 row norm)
    l1 = -log( exp(2 na_i.nb_i) / (sum_j exp(2 na_i.na_j) + sum_j exp(2 na_i.nb_j) - e^2) )
    l2 = same with roles swapped (uses column sums of the cross matrix)
    out = (l1 + l2)/2

Sharding: each core owns a 1024-row block.  The host passes, per core, the
full transposed inputs [256, 8192] with that core's 1024-col block APPENDED
(cols 8192..9215) -> one SPMD NEFF, no dynamic indexing.

Per core: project both tensors (PE matmuls, ELU = min(exp, relu+1) with the
-1 folded into b2 on the host), L2-normalize via PE-ones column sums + DVE
reciprocal + ACT sqrt, then three similarity streams over [block x full]
with fused exp+row-sum on the scalar engine (activation accum_out):
  R1 = (na, na)  -> denom1 refl term       (interleaved with z2 projection)
  B  = (na, nb)  -> denom1 cross term; its exp tiles are also column-summed
                    (PE ones) and ReduceScatter'ed for denom2's cross term
  R2 = (nb, nb)  -> denom2 refl term
The positive-pair term uses log(exp(2 d)) = 2 d directly.
"""

import sys

sys.path.insert(0, "/opt/trn_rl_repo")

import numpy as np
import ml_dtypes

import concourse.bass as bass
import concourse.mybir as mybir
import concourse.tile as tile
from concourse import bacc

BF16 = mybir.dt.bfloat16
F32 = mybir.dt.float32
AF = mybir.ActivationFunctionType
ALU = mybir.AluOpType

P = 128
D = 256
KT = D // P          # 2 k-tiles
N_FULL = 8192
N_CORES = 8
TAU = 0.5
SIM_SCALE = 1.0 / TAU      # 2.0
E2 = float(np.exp(SIM_SCALE))  # exp(2 * ||na||^2) ~ e^2, diag of refl


def build_bass(n_full=N_FULL, blk=None, n_cores=N_CORES):
    """Trace the Tile kernel.  Returns the compiled Bacc object (SPMD)."""
    if blk is None:
        blk = n_full // n_cores
    W = n_full + blk              # projected columns per tensor
    CH = 512                      # projection chunk (free dim per matmul)
    NCH = W // CH
    NBLK = n_full // CH           # chunk index where the block columns start
    ISUB = blk // P               # i-subtiles per core block
    NJ = n_full // 1024           # rs granularity (1024-wide accum columns)

    nc = bacc.Bacc("TRN2", target_bir_lowering=False, debug=False,
                   num_devices=n_cores)

    z1t = nc.dram_tensor("z1t", [D, W], BF16, kind="ExternalInput")
    z2t = nc.dram_tensor("z2t", [D, W], BF16, kind="ExternalInput")
    w1t = nc.dram_tensor("w1t", [D, D], BF16, kind="ExternalInput")
    w2t = nc.dram_tensor("w2t", [D, D], BF16, kind="ExternalInput")
    b1d = nc.dram_tensor("b1", [D], F32, kind="ExternalInput")
    b2d = nc.dram_tensor("b2", [D], F32, kind="ExternalInput")
    out = nc.dram_tensor("out", [P, ISUB], F32, kind="ExternalOutput")

    with tile.TileContext(nc) as tc:
        with (
            tc.tile_pool(name="const", bufs=1) as cpool,
            tc.tile_pool(name="persist", bufs=1) as ppool,
            tc.tile_pool(name="io", bufs=4) as iopool,
            tc.tile_pool(name="scratch", bufs=4) as spool,
            tc.tile_pool(name="dram", bufs=2, space="DRAM") as dpool,
        ):
            # ---- constants ----
            w1_sb = cpool.tile([P, KT, D], BF16)
            nc.sync.dma_start(w1_sb, w1t.rearrange("(k p) c -> p k c", p=P))
            w2_sb = cpool.tile([P, KT, D], BF16)
            nc.sync.dma_start(w2_sb, w2t.rearrange("(k p) c -> p k c", p=P))
            b1f = cpool.tile([P, KT], F32)
            nc.sync.dma_start(b1f, b1d.rearrange("(m p) -> p m", p=P))
            b2f = cpool.tile([P, KT], F32)
            nc.sync.dma_start(b2f, b2d.rearrange("(m p) -> p m", p=P))
            # derived bias forms for the relu path: relu(x+b) = max(x,-b)+b
            nb1 = cpool.tile([P, KT], F32)
            nc.vector.tensor_scalar_mul(nb1, b1f, -1.0)
            b1p1 = cpool.tile([P, KT], F32)
            nc.vector.tensor_scalar_add(b1p1, b1f, 1.0)
            ones_col = cpool.tile([P, 1], BF16)
            nc.vector.memset(ones_col, 1.0)

            rs = ppool.tile([P, 4, ISUB * NJ], F32)
            nc.vector.memset(rs, 0.0)

            cc_in = dpool.tile([n_full], F32, name="cc_in")
            cc_out = dpool.tile([blk], F32, name="cc_out")

            def cs_sink(joff, cs_sb):
                nc.sync.dma_start(cc_in[None, joff:joff + CH], cs_sb[0:1, :])

            def emit_group(st, lhs_b, rhs_b, grp, psS, nbufs, isub, jg,
                           cs_tiles=None):
                """One [128 x grp] similarity tile: matmuls + fused
                exp/row-sum.  With cs_tiles also accumulates column sums
                of the exp tile across isub (for the ReduceScatter)."""
                gb = grp // CH
                lsl = slice(n_full + isub * P, n_full + (isub + 1) * P)
                pg = psS.tile([P, gb, CH], F32, tag="sgrp",
                              bufs=nbufs, name="pg")
                for k in range(KT):
                    for js in range(gb):
                        jss = slice(jg * grp + js * CH,
                                    jg * grp + (js + 1) * CH)
                        nc.tensor.matmul(
                            pg[:, js], lhs_b[:, k, lsl],
                            rhs_b[:, k, jss], start=(k == 0),
                            stop=(k == KT - 1))
                eg = spool.tile([P, gb, CH], BF16, tag="eg")
                col = isub * NJ + (jg * grp) // 1024
                nc.scalar.activation(
                    eg, pg, AF.Exp, scale=SIM_SCALE,
                    accum_out=rs[:, st, col:col + 1])
                if cs_tiles is not None:
                    for js in range(gb):
                        nc.tensor.matmul(cs_tiles[js], ones_col, eg[:, js],
                                         start=(isub == 0),
                                         stop=(isub == ISUB - 1))
                    if isub == ISUB - 1:
                        for js in range(gb):
                            cs_sb = spool.tile([1, CH], F32, tag="ns_sb")
                            nc.vector.tensor_copy(cs_sb, cs_tiles[js])
                            cs_sink(jg * grp + js * CH, cs_sb)

            # ================= projection of z1 then z2 =================
            na_bufs = []
            psR1_cm = psR1 = None
            r1_groups = []
            fin = ppool.tile([P, 10, ISUB], F32)
            for idx, zt in enumerate((z1t, z2t)):
                zt_ap = zt.rearrange("(k p) w -> p k w", p=P)
                naT = ppool.tile([P, KT, W], BF16, name=f"naT{idx}",
                                 tag=f"naT{idx}")
                hT = naT
                nsq = ppool.tile([NCH, CH], F32, name=f"nsq{idx}", tag="nsq")
                rn_dram = dpool.tile([NCH, CH], BF16, name=f"rn{idx}")

                if idx == 1:
                    r1_groups = [(isub, jg) for jg in range(NJ)
                                 for isub in range(ISUB)]
                    psR1_cm = tc.tile_pool(name="psR1", bufs=1, space="PSUM")
                    psR1 = psR1_cm.__enter__()

                # ---- phase A: project, ELU, h->sbuf, squares, col norms ---
                with tc.tile_pool(name="psA", bufs=2, space="PSUM") as psA:
                    for c in range(NCH):
                        cs = slice(c * CH, (c + 1) * CH)
                        zch = iopool.tile([P, KT, CH], BF16, tag="zch")
                        nc.sync.dma_start(zch, zt_ap[:, :, cs])
                        # L1: pa[m] = W1 @ z.T  (biases in the epilogues)
                        pa = psA.tile([P, KT, CH], F32, name="pa",
                                      tag="pa" if idx == 0 else "pp",
                                      bufs=2 if idx == 0 else None)
                        for m in range(KT):
                            ms = slice(m * P, (m + 1) * P)
                            for k in range(KT):
                                nc.tensor.matmul(pa[:, m], w1_sb[:, k, ms],
                                                 zch[:, k], start=(k == 0),
                                                 stop=(k == KT - 1))
                        # ELU' = elu+1 = min(exp(x+b1), relu(x+b1)+1)
                        # (the -1 is folded into b2 on the host)
                        e_t = spool.tile([P, KT, CH], BF16, tag="e")
                        r_t = spool.tile([P, KT, CH], BF16, tag="r")
                        aT = spool.tile([P, KT, CH], BF16, tag="aT")
                        for m in range(KT):
                            nc.scalar.activation(e_t[:, m], pa[:, m], AF.Exp,
                                                 bias=b1f[:, m:m + 1])
                            # relu(x+b1)+1 = (x max -b1) + (b1+1)
                            nc.vector.tensor_scalar(r_t[:, m], pa[:, m],
                                                    nb1[:, m:m + 1],
                                                    b1p1[:, m:m + 1],
                                                    ALU.max, ALU.add)
                            nc.vector.tensor_tensor(aT[:, m], e_t[:, m],
                                                    r_t[:, m], ALU.min)
                        # L2: ph[m2] = W2 @ a
                        ph = psA.tile([P, KT, CH], F32, name="ph",
                                      tag="ph" if idx == 0 else "pp",
                                      bufs=1 if idx == 0 else None)
                        for m2 in range(KT):
                            ms = slice(m2 * P, (m2 + 1) * P)
                            for m in range(KT):
                                nc.tensor.matmul(ph[:, m2], w2_sb[:, m, ms],
                                                 aT[:, m], start=(m == 0),
                                                 stop=(m == KT - 1))
                        # h = ph + b2 -> sbuf bf16 (ACT in the z1 window,
                        # DVE in the z2 window); sq = h*h; norms on PE
                        sq = spool.tile([P, KT, CH], BF16, tag="sq")
                        for m2 in range(KT):
                            if idx == 0:
                                nc.scalar.activation(hT[:, m2, cs], ph[:, m2],
                                                     AF.Identity,
                                                     bias=b2f[:, m2:m2 + 1])
                            else:
                                nc.vector.tensor_scalar(hT[:, m2, cs],
                                                        ph[:, m2],
                                                        b2f[:, m2:m2 + 1],
                                                        None, ALU.add)
                            nc.vector.tensor_tensor(sq[:, m2], hT[:, m2, cs],
                                                    hT[:, m2, cs], ALU.mult)
                        ns = psA.tile([1, CH], F32, name="ns",
                                      tag="ns" if idx == 0 else "pp",
                                      bufs=2 if idx == 0 else None)
                        for m2 in range(KT):
                            nc.tensor.matmul(ns, ones_col, sq[:, m2],
                                             start=(m2 == 0),
                                             stop=(m2 == KT - 1))
                        ns_sb = spool.tile([1, CH], F32, tag="ns_sb")
                        if idx == 0:
                            nc.scalar.copy(ns_sb, ns)
                        else:
                            nc.vector.tensor_copy(ns_sb, ns)
                        nc.sync.dma_start(nsq[c:c + 1, :], ns_sb)
                        # interleave ready R1 groups (naT0 complete by now)
                        if idx == 1:
                            take, r1_groups = r1_groups[:4], r1_groups[4:]
                            for isub, jg in take:
                                emit_group(0, na_bufs[0], na_bufs[0], 1024,
                                           psR1, 2, isub, jg)
                if idx == 1:
                    for isub, jg in r1_groups:
                        emit_group(0, na_bufs[0], na_bufs[0], 1024,
                                   psR1, 2, isub, jg)

                # ---- phase B: rn = nsq^-1/2 = exp(-0.5 ln nsq) ----
                # (stays in the natural_log_exp ACT table set: no swaps)
                lns = ppool.tile([NCH, CH], F32, name=f"lns{idx}", tag="lns")
                nc.scalar.activation(lns, nsq, AF.Ln)
                rn_sb = ppool.tile([NCH, CH], BF16, name=f"rnsb{idx}",
                                   tag="rnsb")
                nc.scalar.activation(rn_sb, lns, AF.Exp, scale=-0.5)
                nc.sync.dma_start(rn_dram, rn_sb)

                # ---- phase C: naT = hT * rn (in place), block cols first;
                # z2's phase C interleaves pos + stream-B groups.
                if idx == 0:
                    for c in list(range(NBLK, NCH)) + list(range(NBLK)):
                        cs = slice(c * CH, (c + 1) * CH)
                        rnB = spool.tile([P, CH], BF16, tag="rnB")
                        nc.sync.dma_start(
                            rnB, rn_dram[c:c + 1, :].to_broadcast([P, CH]))
                        for k in range(KT):
                            nc.vector.tensor_tensor(naT[:, k, cs],
                                                    hT[:, k, cs],
                                                    rnB, ALU.mult)
                na_bufs.append(naT)

            naT, nbT = na_bufs
            rn_dram2 = rn_dram

            def do_c2_chunk(c):
                cs = slice(c * CH, (c + 1) * CH)
                rnB = spool.tile([P, CH], BF16, tag="rnB")
                nc.sync.dma_start(
                    rnB, rn_dram2[c:c + 1, :].to_broadcast([P, CH]))
                for k in range(KT):
                    nc.vector.tensor_tensor(nbT[:, k, cs], nbT[:, k, cs],
                                            rnB, ALU.mult)

            def do_pos():
                pd = spool.tile([P, KT, blk], BF16, tag="pd")
                for k in range(KT):
                    nc.vector.tensor_tensor(pd[:, k], naT[:, k, n_full:],
                                            nbT[:, k, n_full:], ALU.mult)
                pos_ps = psR1.tile([P, ISUB], F32, name="pos_ps", bufs=2,
                                   tag="sgrp")
                for s in range(ISUB):
                    ss = slice(s * P, (s + 1) * P)
                    for k in range(KT):
                        nc.tensor.matmul(pos_ps[:, s:s + 1], pd[:, k, ss],
                                         ones_col, start=(k == 0),
                                         stop=(k == KT - 1))
                nc.vector.tensor_copy(fin[:, 9], pos_ps)

            # C2 block columns first, then pos (psR1 still open for it)
            for c in range(NBLK, NCH):
                do_c2_chunk(c)
            do_pos()
            psR1_cm.__exit__(None, None, None)

            # ---- stream B paced by C2 chunks, + column sums + RS ----
            psB_cm = tc.tile_pool(name="psB", bufs=1, space="PSUM")
            psB = psB_cm.__enter__()
            cs_tiles = [psB.tile([1, CH], F32, name=f"cst{js}",
                                 tag=f"cst{js}", bufs=1)
                        for js in range(2)]
            for jg in range(NJ):
                do_c2_chunk(2 * jg)
                do_c2_chunk(2 * jg + 1)
                for isub in range(ISUB):
                    emit_group(1, naT, nbT, 1024, psB, 2, isub, jg,
                               cs_tiles=cs_tiles)
            nc.gpsimd.collective_compute(
                "ReduceScatter", ALU.add,
                replica_groups=[list(range(n_cores))],
                ins=[cc_in[:]], outs=[cc_out[:]])
            psB_cm.__exit__(None, None, None)

            # ---- stream R2 ----
            with tc.tile_pool(name="psS", bufs=1, space="PSUM") as psS:
                for isub in range(ISUB):
                    for jg in range(n_full // 2048):
                        emit_group(3, nbT, nbT, 2048, psS, 2, isub, jg)

            ccv = ppool.tile([P, ISUB], F32)
            nc.sync.dma_start(ccv, cc_out.rearrange("(s p) -> p s", p=P))

            # ---- final: l = 0.5*(ln d1 + ln d2) - 2*pos ----
            rs4 = rs.rearrange("p s (i j) -> p s i j", j=NJ)
            for st in (0, 1, 3):
                nc.vector.tensor_reduce(out=fin[:, st, :, None],
                                        in_=rs4[:, st], op=ALU.add,
                                        axis=mybir.AxisListType.X)
            # d1 = rsR1 + rsB - e2 ; d2 = rsR2 + colsum_B - e2
            nc.vector.scalar_tensor_tensor(fin[:, 4], fin[:, 0], -E2,
                                           fin[:, 1], ALU.add, ALU.add)
            nc.vector.scalar_tensor_tensor(fin[:, 5], fin[:, 3], -E2,
                                           ccv, ALU.add, ALU.add)
            nc.scalar.activation(fin[:, 6], fin[:, 4], AF.Ln)
            nc.scalar.activation(fin[:, 7], fin[:, 5], AF.Ln)
            nc.vector.tensor_tensor(fin[:, 8], fin[:, 6], fin[:, 7], ALU.add)
            # l = 0.5 * (lnd1 + lnd2 - 2*SIM_SCALE*pos)
            lres = ppool.tile([P, ISUB], F32)
            nc.vector.scalar_tensor_tensor(lres, fin[:, 9],
                                           -2.0 * SIM_SCALE, fin[:, 8],
                                           ALU.mult, ALU.add)
            nc.vector.tensor_scalar_mul(lres, lres, 0.5)
            nc.sync.dma_start(out[:, :], lres)

    nc.compile()
    return nc


def prep_inputs(z1, z2, W1, b1, W2, b2, n_full=N_FULL, n_cores=N_CORES):
    """Host-side prep -> list of per-core input maps (numpy)."""
    blk = n_full // n_cores
    bf = ml_dtypes.bfloat16
    z1t = np.ascontiguousarray(z1.T)
    z2t = np.ascontiguousarray(z2.T)
    w1t = np.ascontiguousarray(W1.T).astype(bf)
    w2t = np.ascontiguousarray(W2.T).astype(bf)
    # ELU' = elu + 1 is used as the L1 activation; fold the "-1" into b2:
    # h = W2 @ (elu'(x) - 1) + b2 = W2 @ elu'(x) + (b2 - W2.sum(1))
    b2_eff = (b2 - W2.sum(axis=1)).astype(np.float32)
    b1c = b1.astype(np.float32)
    in_maps = []
    for c in range(n_cores):
        bs = slice(c * blk, (c + 1) * blk)
        in_maps.append({
            "z1t": np.concatenate([z1t, z1t[:, bs]], axis=1).astype(bf),
            "z2t": np.concatenate([z2t, z2t[:, bs]], axis=1).astype(bf),
            "w1t": w1t, "w2t": w2t, "b1": b1c, "b2": b2_eff,
        })
    return in_maps


_NC_CACHE = {}


def _get_nc(n_full=N_FULL, n_cores=N_CORES):
    key = (n_full, n_cores)
    if key not in _NC_CACHE:
        _NC_CACHE[key] = build_bass(n_full=n_full, n_cores=n_cores)
    return _NC_CACHE[key]


def kernel(z1, z2, W1, b1, W2, b2):
    from concourse.bass_utils import run_bass_kernel_spmd

    n_full = z1.shape[0]
    n_cores = N_CORES
    in_maps = prep_inputs(z1, z2, W1, b1, W2, b2, n_full, n_cores)
    nc = _get_nc(n_full, n_cores)
    res = run_bass_kernel_spmd(nc, in_maps, core_ids=list(range(n_cores)))
    parts = [np.asarray(res.results[c]["out"]).T.reshape(-1)
             for c in range(n_cores)]
    return np.concatenate(parts).astype(np.float32)


if __name__ == "__main__":
    nc = build_bass()
    print("traced ok")
